# revision 1
# baseline (speedup 1.0000x reference)
"""Trainium2 Bass kernel for a dense transformer block (pre-LN, MHA + GELU MLP).

Sharding: 8 cores = 2 batches x 4 sequence-quarters. Each core recomputes
LN1 + K/V for its full batch (zero cross-core communication), and computes
Q/attention/proj/MLP for its own 512 tokens only.

Device works feature-major ([feature, token]); the host pre-transposes x and
post-transposes the output. LN gains/biases are folded into the following
matmul weights on the host; the qk scale (1/8) is folded into W_q; the v bias
is folded into b_proj.

Numerics: matmul operands are bf16 (fp32 PSUM accumulation); the residual
stream (x, x2, out), layernorm statistics, and softmax denominators stay fp32.
LN-statistic / broadcast matmuls run in fp32r.
"""
import sys

sys.path.insert(0, "/opt/trn_rl_repo")

import numpy as np
import ml_dtypes

import concourse.bass as bass  # noqa: F401
import concourse.tile as tile
from concourse import bacc, mybir, bass_utils

F32 = mybir.dt.float32
F32R = mybir.dt.float32r
BF16 = mybir.dt.bfloat16
AF = mybir.ActivationFunctionType
ALU = mybir.AluOpType

P = 128
D = 768
NH = 12
DH = 64
DFF = 3072
TB = 2048      # tokens per batch
TO = 512       # tokens owned per core
NJ = D // P    # 6 feature tiles
NT = TB // TO  # 4 token tiles per batch
NTK = TB // P  # 16 key tiles
NMLP = DFF // P  # 24
EPS = 1e-6
N_CORES = 8
VW = 66        # 64 v cols + 2 ones cols per head


def R(ap):
    return ap.bitcast(F32R)


def _build():
    nc = bacc.Bacc("TRN2", target_bir_lowering=False, debug=False,
                   num_devices=N_CORES)

    x_fm = nc.dram_tensor("x_fm", [D, TB], BF16, kind="ExternalInput").ap()
    x_own_d = nc.dram_tensor("x_own", [D, TO], F32, kind="ExternalInput").ap()
    wqkv = nc.dram_tensor("wqkv", [D, 3 * D], BF16, kind="ExternalInput").ap()
    bqk = nc.dram_tensor("bqk", [P, 12], F32, kind="ExternalInput").ap()
    wproj = nc.dram_tensor("wproj", [D, D], BF16, kind="ExternalInput").ap()
    bproj = nc.dram_tensor("bproj", [P, NJ], F32, kind="ExternalInput").ap()
    wfc1 = nc.dram_tensor("wfc1", [D, DFF], BF16, kind="ExternalInput").ap()
    bfc1 = nc.dram_tensor("bfc1", [P, NMLP], F32, kind="ExternalInput").ap()
    wfc2 = nc.dram_tensor("wfc2", [DFF, D], BF16, kind="ExternalInput").ap()
    bfc2 = nc.dram_tensor("bfc2", [P, NJ], F32, kind="ExternalInput").ap()
    out_fm = nc.dram_tensor("out_fm", [D, TO], F32, kind="ExternalOutput").ap()

    with nc.allow_low_precision(reason="bf16 matmul operands are intentional"), \
            tile.TileContext(nc) as tc:
        _emit(tc, nc, x_fm, x_own_d, wqkv, bqk, wproj, bproj, wfc1, bfc1,
              wfc2, bfc2, out_fm)
    nc.compile()
    return nc


def _ln_batched(nc, pools, x_tiles_by_nt, xn_out_fn, ones_t, half2, eps2,
                n_nt, bf16_in):
    """LayerNorm over n_nt token tiles of 512, one batched stats chain.

    x_tiles_by_nt[nt][j]: input tiles [128, 512] (bf16 if bf16_in else fp32).
    xn_out_fn(nt, j) -> bf16 dest AP [128, 512].
    """
    tc, stats, sq_pool, ln_ps, bc_ps = pools
    ntot = 512 * n_nt
    sum_sb = stats.tile([2, ntot], F32, tag="sum_sb", name="sum_sb")
    sq_sb = stats.tile([2, ntot], F32, tag="sq_sb", name="sq_sb")
    mk = (lambda ap: ap) if bf16_in else R
    sqdt = BF16 if bf16_in else F32
    for nt in range(n_nt):
        x_tiles = x_tiles_by_nt[nt]
        xsq = []
        for j in range(NJ):
            t = sq_pool.tile([P, 512], sqdt, tag=f"xsq{j}", name="xsqt")
            nc.scalar.activation(out=mk(t), in_=x_tiles[j], func=AF.Square)
            xsq.append(t)
        ps_sum = ln_ps.tile([2, 512], F32, tag="lnsum", name="ps_sum")
        ps_sq = ln_ps.tile([2, 512], F32, tag="lnsq", name="ps_sq")
        for j in range(NJ):
            nc.tensor.matmul(ps_sum[:], lhsT=mk(ones_t), rhs=mk(x_tiles[j]),
                             start=(j == 0), stop=(j == NJ - 1))
        for j in range(NJ):
            nc.tensor.matmul(ps_sq[:], lhsT=mk(ones_t), rhs=mk(xsq[j]),
                             start=(j == 0), stop=(j == NJ - 1))
        sl = slice(nt * 512, (nt + 1) * 512)
        nc.vector.tensor_copy(out=sum_sb[:, sl], in_=ps_sum)
        nc.vector.tensor_copy(out=sq_sb[:, sl], in_=ps_sq)
    # var*D^2 = D*sumsq - sum^2 ; rs = exp(-0.5*ln(varD2/D^2 + eps))
    t1 = stats.tile([2, ntot], F32, tag="t1", name="t1")
    nc.vector.scalar_tensor_tensor(out=t1, in0=sum_sb, scalar=-1.0,
                                   in1=sum_sb, op0=ALU.mult, op1=ALU.mult)
    nc.vector.scalar_tensor_tensor(out=t1, in0=sq_sb, scalar=float(D),
                                   in1=t1, op0=ALU.mult, op1=ALU.add)
    nc.scalar.activation(out=t1, in_=t1, func=AF.Ln, bias=eps2,
                         scale=1.0 / (D * D))
    rs = stats.tile([2, ntot], F32, tag="rs", name="rs")
    nc.scalar.activation(out=R(rs), in_=t1, func=AF.Exp, scale=-0.5)
    cc = stats.tile([2, ntot], F32, tag="cc", name="cc")
    nc.vector.scalar_tensor_tensor(out=R(cc), in0=sum_sb, scalar=-1.0 / D,
                                   in1=rs, op0=ALU.mult, op1=ALU.mult)
    for nt in range(n_nt):
        sl = slice(nt * 512, (nt + 1) * 512)
        ps_a = bc_ps.tile([P, 512], F32, tag="bca", name="ps_a")
        nc.tensor.matmul(ps_a[:], lhsT=R(half2), rhs=R(rs[:, sl]),
                         start=True, stop=True)
        ps_c = bc_ps.tile([P, 512], F32, tag="bcc", name="ps_c")
        nc.tensor.matmul(ps_c[:], lhsT=R(half2), rhs=R(cc[:, sl]),
                         start=True, stop=True)
        for j in range(NJ):
            tmp = sq_pool.tile([P, 512], F32, tag=f"tmp{j}", name="xnt")
            nc.vector.tensor_mul(out=tmp, in0=x_tiles_by_nt[nt][j], in1=ps_a)
            nc.vector.tensor_add(out=xn_out_fn(nt, j), in0=tmp, in1=ps_c)


def _emit(tc, nc, x_fm, x_own_d, wqkv, bqk, wproj_d, bproj_d, wfc1_d, bfc1_d,
          wfc2_d, bfc2_d, out_fm):
    ctx_pools = []

    cons_pool = tc.alloc_tile_pool(name="cons", bufs=1)
    ctx_pools.append(cons_pool)
    ones2 = cons_pool.tile([P, 2], F32)
    nc.vector.memset(ones2, 1.0)
    ones2b = cons_pool.tile([P, 2], BF16)
    nc.vector.memset(ones2b, 1.0)
    half2 = cons_pool.tile([2, P], F32)
    nc.vector.memset(half2, 0.5)
    eps2 = cons_pool.tile([2, 1], F32)
    nc.vector.memset(eps2, EPS)

    bqk_sb = cons_pool.tile([P, 12], F32)
    nc.sync.dma_start(out=bqk_sb, in_=bqk)
    bproj_sb = cons_pool.tile([P, NJ], F32)
    nc.sync.dma_start(out=bproj_sb, in_=bproj_d)
    bfc1_sb = cons_pool.tile([P, NMLP], F32)
    nc.sync.dma_start(out=bfc1_sb, in_=bfc1_d)
    bfc2_sb = cons_pool.tile([P, NJ], F32)
    nc.sync.dma_start(out=bfc2_sb, in_=bfc2_d)

    stats = tc.alloc_tile_pool(name="stats", bufs=2)
    ctx_pools.append(stats)

    # k/q bf16; x_own fp32 residual; live until proj.
    persist = tc.alloc_tile_pool(name="persist", bufs=1)
    k_sb = [persist.tile([P, TB], BF16, tag=f"k{j}", name=f"k{j}")
            for j in range(NJ)]
    q_sb = [persist.tile([P, TO], BF16, tag=f"q{j}", name=f"q{j}")
            for j in range(NJ)]
    x_own = [persist.tile([P, TO], F32, tag=f"xo{j}", name=f"xo{j}")
             for j in range(NJ)]

    v_pool = tc.alloc_tile_pool(name="vpool", bufs=1, side="right")
    v_sb = [v_pool.tile([P, NH * VW], BF16, tag=f"v{t}", name=f"v{t}")
            for t in range(NTK)]

    xn_pool = tc.alloc_tile_pool(name="xnpool", bufs=1)
    xn_all = [xn_pool.tile([P, TB], BF16, tag=f"xn{j}", name=f"xn{j}")
              for j in range(NJ)]

    # ---------------- Phase 1: load x (bf16), LN1 -> xn_all (bf16) ---------
    with (
        tc.tile_pool(name="xstream", bufs=1) as xpool,
        tc.tile_pool(name="sqpool", bufs=2) as sq_pool,
        tc.tile_pool(name="lnps", bufs=2, space="PSUM") as ln_ps,
        tc.tile_pool(name="bcps", bufs=2, space="PSUM") as bc_ps,
    ):
        for j in range(NJ):
            nc.sync.dma_start(out=x_own[j], in_=x_own_d[j * P:(j + 1) * P, :])
        pools = (tc, stats, sq_pool, ln_ps, bc_ps)
        for nt in range(NT):
            xt = [xpool.tile([P, TO], BF16, tag=f"xs{nt}_{j}",
                             name=f"xs{nt}_{j}") for j in range(NJ)]
            for j in range(NJ):
                nc.sync.dma_start(
                    out=xt[j],
                    in_=x_fm[j * P:(j + 1) * P, nt * TO:(nt + 1) * TO])
            _ln_batched(nc, pools, [xt],
                        lambda n_, j, nt=nt: xn_all[j][:, nt * TO:(nt + 1) * TO],
                        ones2b, half2, eps2, 1, True)

    # ---------------- Phase 2: Q, V, then K (bf16) ----------------
    # V is emitted before K so that attention (gated on K) starts only after
    # V is resident; av accumulation then never convoys behind the V matmuls.
    with (
        tc.tile_pool(name="wkq", bufs=1) as wkq_pool,
        tc.tile_pool(name="mmps", bufs=4, space="PSUM") as mm_ps,
        tc.tile_pool(name="wv", bufs=1) as wv_pool,
        tc.tile_pool(name="vps5", bufs=2, space="PSUM") as v_ps5,
    ):
        wkq = []
        for j in range(NJ):
            t = wkq_pool.tile([P, 2 * D], BF16, tag=f"wkq{j}", name=f"wkq{j}")
            nc.sync.dma_start(out=t, in_=wqkv[j * P:(j + 1) * P, 0:2 * D])
            wkq.append(t)
        wv = []
        for j in range(NJ):
            t = wv_pool.tile([P, D], BF16, tag=f"wv{j}", name=f"wv{j}")
            nc.sync.dma_start(out=t, in_=wqkv[j * P:(j + 1) * P, 2 * D:3 * D])
            wv.append(t)
        # Q for own tokens
        for m in range(NJ):
            pt = mm_ps.tile([P, TO], F32, tag="mm", name="mmq")
            for j in range(NJ):
                nc.tensor.matmul(pt[:], lhsT=wkq[j][:, m * P:(m + 1) * P],
                                 rhs=xn_all[j][:, 0:TO],
                                 start=(j == 0), stop=(j == NJ - 1))
            nc.vector.tensor_scalar_add(q_sb[m], pt, bqk_sb[:, m:m + 1])
        # K for all tokens
        for m in range(NJ):
            for nt in range(NT):
                pt = mm_ps.tile([P, TO], F32, tag="mm", name="mmk")
                for j in range(NJ):
                    nc.tensor.matmul(
                        pt[:], lhsT=wkq[j][:, D + m * P:D + (m + 1) * P],
                        rhs=xn_all[j][:, nt * TO:(nt + 1) * TO],
                        start=(j == 0), stop=(j == NJ - 1))
                nc.vector.tensor_scalar_add(
                    k_sb[m][:, nt * TO:(nt + 1) * TO], pt,
                    bqk_sb[:, 6 + m:7 + m])
        # V (token-major with ones columns)
        for mt in range(NTK):
            vt = v_sb[mt]
            nc.vector.memset(
                vt.rearrange("p (h w) -> p h w", w=VW)[:, :, 64:66], 1.0)
            pt5 = v_ps5.tile([P, 512], F32, tag="v5", name="v5")
            pt2 = mm_ps.tile([P, TO], F32, tag="mm", name="v2")
            for j in range(NJ):
                lhs = xn_all[j][:, mt * P:(mt + 1) * P]
                nc.tensor.matmul(pt5[:], lhsT=lhs, rhs=wv[j][:, 0:512],
                                 start=(j == 0), stop=(j == NJ - 1))
            for j in range(NJ):
                lhs = xn_all[j][:, mt * P:(mt + 1) * P]
                nc.tensor.matmul(pt2[:, 0:256], lhsT=lhs, rhs=wv[j][:, 512:768],
                                 start=(j == 0), stop=(j == NJ - 1))
            v3 = vt.rearrange("p (h w) -> p h w", w=VW)
            nc.vector.tensor_copy(
                out=v3[:, 0:8, 0:64],
                in_=pt5.rearrange("p (h w) -> p h w", w=64))
            nc.vector.tensor_copy(
                out=v3[:, 8:12, 0:64],
                in_=pt2[:, 0:256].rearrange("p (h w) -> p h w", w=64))
    xn_pool.release()

    # ---------------- Phase 4: attention ----------------
    attn_pool = tc.alloc_tile_pool(name="attnpool", bufs=1)
    attn_fm = [attn_pool.tile([P, TO], BF16, tag=f"at{j}", name=f"at{j}")
               for j in range(NJ)]
    av_sb = [attn_pool.tile([64, TO], F32, tag=f"av{h}", name=f"av{h}")
             for h in range(NH)]
    rec12 = [attn_pool.tile([2, 512], F32, tag=f"rc{h}", name=f"rc{h}")
             for h in range(NH)]
    wp_pool = tc.alloc_tile_pool(name="wproj", bufs=1)
    wp = []
    for j in range(NJ):
        t = wp_pool.tile([P, D], BF16, tag=f"wp{j}", name=f"wp{j}")
        nc.sync.dma_start(out=t, in_=wproj_d[j * P:(j + 1) * P, :])
        wp.append(t)
    with (
        tc.tile_pool(name="seps", bufs=3, space="PSUM") as se_ps,
        tc.tile_pool(name="avps", bufs=1, space="PSUM") as av_ps,
        tc.tile_pool(name="sesb", bufs=6) as se_pool,
        tc.tile_pool(name="bcsb", bufs=2) as bc_pool,
    ):
        for hp in range(NJ):
            pt_av_a = av_ps.tile([P, 512], F32, tag="ava", name="ava")
            pt_av_b = av_ps.tile([P, 512], F32, tag="avb", name="avb")
            for tk2 in range(NTK // 2):
                ps_a = se_ps.tile([P, 1024], F32, tag="se", name="psea")
                ps_b = se_ps.tile([P, 1024], F32, tag="se", name="pseb")
                for half in range(2):
                    tk = 2 * tk2 + half
                    ksl = slice(tk * P, (tk + 1) * P)
                    fsl = slice(half * 512, (half + 1) * 512)
                    nc.tensor.matmul(ps_a[:, fsl],
                                     lhsT=k_sb[hp][0:64, ksl],
                                     rhs=q_sb[hp][0:64, :],
                                     start=True, stop=True)
                    nc.tensor.matmul(ps_b[:, fsl],
                                     lhsT=k_sb[hp][64:128, ksl],
                                     rhs=q_sb[hp][64:128, :],
                                     start=True, stop=True)
                se_a = se_pool.tile([P, 1024], BF16, tag="sea", name="sea")
                se_b = se_pool.tile([P, 1024], BF16, tag="seb", name="seb")
                nc.scalar.activation(out=se_a, in_=ps_a, func=AF.Exp)
                nc.scalar.activation(out=se_b, in_=ps_b, func=AF.Exp)
                for half in range(2):
                    tk = 2 * tk2 + half
                    fsl = slice(half * 512, (half + 1) * 512)
                    first = (tk == 0)
                    last = (tk == NTK - 1)
                    nc.tensor.matmul(
                        pt_av_a[:VW, :],
                        lhsT=v_sb[tk][:, (2 * hp) * VW:(2 * hp + 1) * VW],
                        rhs=se_a[:, fsl], start=first, stop=last)
                    nc.tensor.matmul(
                        pt_av_b[:VW, :],
                        lhsT=v_sb[tk][:, (2 * hp + 1) * VW:(2 * hp + 2) * VW],
                        rhs=se_b[:, fsl], start=first, stop=last)
            for head, pt_av in ((0, pt_av_a), (1, pt_av_b)):
                # Evacuate numerator + reciprocal now (DVE is idle during the
                # ACT-bound exp stream); broadcast/divide deferred so no PSUM
                # slot is held across the slow reciprocal.
                h = 2 * hp + head
                nc.vector.tensor_copy(out=av_sb[h][0:64, :],
                                      in_=pt_av[0:64, :])
                nc.vector.reciprocal(out=R(rec12[h]), in_=pt_av[64:66, :])
    # Division tails on evacuated data, fresh PSUM pool.
    with (
        tc.tile_pool(name="divps", bufs=4, space="PSUM") as div_ps,
        tc.tile_pool(name="divsb", bufs=4) as div_pool,
    ):
        for h in range(NH):
            ps_bc = div_ps.tile([64, 512], F32, tag="bc", name="psbc")
            nc.tensor.matmul(ps_bc[:], lhsT=R(half2[:, 0:64]), rhs=R(rec12[h]),
                             start=True, stop=True)
            bc_sb = div_pool.tile([64, 512], F32, tag="bc", name="bcsb")
            nc.vector.tensor_copy(out=bc_sb, in_=ps_bc)
            hp, head = divmod(h, 2)
            nc.vector.tensor_mul(
                out=attn_fm[hp][head * 64:(head + 1) * 64, :],
                in0=av_sb[h][0:64, :], in1=bc_sb)
    v_pool.release()

    # ---------------- Phase 5: proj + residual -> x2 (fp32) ----------------
    x2_pool = tc.alloc_tile_pool(name="x2pool", bufs=1, side="right")
    ctx_pools.append(x2_pool)
    x2_sb = [x2_pool.tile([P, TO], F32, tag=f"x2{j}", name=f"x2{j}")
             for j in range(NJ)]
    wfc1_pool = tc.alloc_tile_pool(name="wfc1", bufs=1, side="right")
    ctx_pools.append(wfc1_pool)
    wf1 = []
    for j in range(NJ):
        t = wfc1_pool.tile([P, DFF], BF16, tag=f"wf1{j}", name=f"wf1{j}")
        nc.sync.dma_start(out=t, in_=wfc1_d[j * P:(j + 1) * P, :])
        wf1.append(t)
    with (
        tc.tile_pool(name="mmps2", bufs=3, space="PSUM") as mm_ps2,
    ):
        for m in range(NJ):
            pt = mm_ps2.tile([P, TO], F32, tag="mm", name="mmproj")
            for j in range(NJ):
                nc.tensor.matmul(pt[:], lhsT=wp[j][:, m * P:(m + 1) * P],
                                 rhs=attn_fm[j],
                                 start=(j == 0), stop=(j == NJ - 1))
            nc.vector.scalar_tensor_tensor(
                out=R(x2_sb[m]), in0=pt, scalar=bproj_sb[:, m:m + 1],
                in1=x_own[m], op0=ALU.add, op1=ALU.add)
    wp_pool.release()
    attn_pool.release()
    persist.release()

    # ---------------- Phase 6: LN2 -> h (bf16) ----------------
    h_pool = tc.alloc_tile_pool(name="hpool", bufs=1, side="right")
    ctx_pools.append(h_pool)
    h_sb = [h_pool.tile([P, TO], BF16, tag=f"h{j}", name=f"h{j}")
            for j in range(NJ)]
    with (
        tc.tile_pool(name="sqpool2", bufs=2) as sq_pool2,
        tc.tile_pool(name="lnps2", bufs=1, space="PSUM") as ln_ps2,
        tc.tile_pool(name="bcps3", bufs=1, space="PSUM") as bc_ps3,
    ):
        pools = (tc, stats, sq_pool2, ln_ps2, bc_ps3)
        _ln_batched(nc, pools, [x2_sb],
                    lambda nt, j: h_sb[j][:, :],
                    ones2, half2, eps2, 1, False)

    # ---------------- Phase 7: fc1 + gelu -> h1 (bf16) ----------------
    h1_pool = tc.alloc_tile_pool(name="h1", bufs=1, side="right")
    ctx_pools.append(h1_pool)
    h1_sb = [h1_pool.tile([P, TO], BF16, tag=f"h1{m}", name=f"h1{m}")
             for m in range(NMLP)]
    with (
        tc.tile_pool(name="mmps3", bufs=4, space="PSUM") as mm_ps3,
    ):
        for m in range(NMLP):
            pt = mm_ps3.tile([P, TO], F32, tag="mm", name="mmfc1")
            for j in range(NJ):
                nc.tensor.matmul(pt[:], lhsT=wf1[j][:, m * P:(m + 1) * P],
                                 rhs=h_sb[j],
                                 start=(j == 0), stop=(j == NJ - 1))
            nc.scalar.activation(out=h1_sb[m], in_=pt, func=AF.Gelu,
                                 bias=bfc1_sb[:, m:m + 1])

    # ---------------- Phase 8: fc2 + residual + store ----------------
    with (
        tc.tile_pool(name="wfc2", bufs=6) as wfc2_pool,
        tc.tile_pool(name="fc2ps", bufs=1, space="PSUM") as fc2_ps,
        tc.tile_pool(name="outsb", bufs=2) as out_pool,
    ):
        pts = [fc2_ps.tile([P, TO], F32, tag=f"fc2_{m}", name=f"fc2_{m}")
               for m in range(NJ)]
        for j in range(NMLP):
            wt = wfc2_pool.tile([P, D], BF16, tag="wf2", name="wf2")
            nc.sync.dma_start(out=wt, in_=wfc2_d[j * P:(j + 1) * P, :])
            for m in range(NJ):
                nc.tensor.matmul(pts[m][:], lhsT=wt[:, m * P:(m + 1) * P],
                                 rhs=h1_sb[j],
                                 start=(j == 0), stop=(j == NMLP - 1))
        for m in range(NJ):
            ot = out_pool.tile([P, TO], F32, tag="out", name="ot")
            nc.vector.scalar_tensor_tensor(
                out=ot, in0=pts[m], scalar=bfc2_sb[:, m:m + 1],
                in1=x2_sb[m], op0=ALU.add, op1=ALU.add)
            nc.sync.dma_start(out=out_fm[m * P:(m + 1) * P, :], in_=ot)

    for pool in reversed(ctx_pools):
        pool.release()


_NC_CACHE = {}


def _get_nc():
    if "nc" not in _NC_CACHE:
        _NC_CACHE["nc"] = _build()
    return _NC_CACHE["nc"]


def _host_prep(inputs):
    f32 = lambda a: np.ascontiguousarray(np.asarray(a, dtype=np.float32))
    x = f32(inputs["x"])            # [2, 2048, 768]
    W_qkv = f32(inputs["W_qkv"])    # [768, 2304]
    b_qkv = f32(inputs["b_qkv"])
    W_proj = f32(inputs["W_proj"])
    b_proj = f32(inputs["b_proj"])
    W_fc1 = f32(inputs["W_fc1"])
    b_fc1 = f32(inputs["b_fc1"])
    W_fc2 = f32(inputs["W_fc2"])
    b_fc2 = f32(inputs["b_fc2"])
    ln1_g = f32(inputs["ln1_g"])
    ln1_b = f32(inputs["ln1_b"])
    ln2_g = f32(inputs["ln2_g"])
    ln2_b = f32(inputs["ln2_b"])

    scale = DH ** -0.5
    wqkv_eff = W_qkv * ln1_g[:, None]
    bqkv_eff = ln1_b @ W_qkv + b_qkv
    wqkv_eff[:, :D] *= scale
    bqkv_eff_q = bqkv_eff[:D] * scale
    bqk = np.concatenate([bqkv_eff_q, bqkv_eff[D:2 * D]]).astype(np.float32)
    bv = bqkv_eff[2 * D:]
    bproj_eff = (b_proj + bv @ W_proj).astype(np.float32)
    wfc1_eff = (W_fc1 * ln2_g[:, None]).astype(np.float32)
    bfc1_eff = (ln2_b @ W_fc1 + b_fc1).astype(np.float32)

    bf = lambda a: np.ascontiguousarray(a.astype(ml_dtypes.bfloat16))
    pack = lambda b: np.ascontiguousarray(
        b.reshape(-1, P).T.astype(np.float32))
    shared = {
        "wqkv": bf(wqkv_eff),
        "bqk": pack(bqk),
        "wproj": bf(W_proj),
        "bproj": pack(bproj_eff),
        "wfc1": bf(wfc1_eff),
        "bfc1": pack(bfc1_eff),
        "wfc2": bf(W_fc2),
        "bfc2": pack(b_fc2),
    }
    in_maps = []
    for c in range(N_CORES):
        b, q = divmod(c, 4)
        xb = np.roll(x[b], -TO * q, axis=0)  # own tokens at rows 0:TO
        m = dict(shared)
        m["x_fm"] = bf(xb.T)
        m["x_own"] = np.ascontiguousarray(xb[:TO].T)
        in_maps.append(m)
    return in_maps


def _run(inputs, trace=False):
    nc = _get_nc()
    in_maps = _host_prep(inputs)
    res = bass_utils.run_bass_kernel_spmd(nc, in_maps, list(range(N_CORES)),
                                          trace=trace)
    B = 2
    out = np.empty((B, TB, D), dtype=np.float32)
    for c in range(N_CORES):
        b, q = divmod(c, 4)
        out[b, TO * q:TO * (q + 1), :] = res.results[c]["out_fm"].T
    return out, res


def kernel(**inputs):
    out, _ = _run(inputs, trace=False)
    return out


if __name__ == "__main__":
    print("building...")
    _get_nc()
    print("built ok")



# revision 20
# speedup vs baseline: 1.3034x; 1.3034x over previous
"""Trainium2 Bass kernel for a dense transformer block (pre-LN, MHA + GELU MLP).

Sharding: 8 cores = 2 batches x 4 sequence-quarters. Each core recomputes
LN1 + K/V for its full batch (zero cross-core communication), and computes
Q/attention/proj/MLP for its own 512 tokens only.

Device works feature-major ([feature, token]); the host pre-transposes x and
post-transposes the output. LN gains/biases are folded into the following
matmul weights on the host; the qk scale (1/8) is folded into W_q; the v bias
is folded into b_proj.

Numerics: matmul operands are bf16 (fp32 PSUM accumulation); the residual
stream (x2, out), layernorm statistics, and softmax denominators stay fp32.
The attention residual uses the bf16 x tiles (error well inside tolerance).

Schedule notes (v2): all x DMAs are issued before any weight DMA; LN1 stats
are reduced per 512-token tile as DMAs land but the Ln/Exp rsqrt chain runs
once for all 2048 tokens (the act-table pass is greedy per-function, so each
Ln/Exp pair costs two ~1.3us table loads - batching keeps it to one pair).
Q/K matmul emission is interleaved with the LN1 broadcast/apply steps so the
PE queue never drains (PE drops to a 1.2GHz p-state after idling). Softmax
denominators are reciprocated as one batched exp(-ln(x)) on ACT instead of
per-head DVE reciprocal (3.3us each there).
"""
import sys

sys.path.insert(0, "/opt/trn_rl_repo")

import numpy as np
import ml_dtypes

import concourse.bass as bass  # noqa: F401
import concourse.tile as tile
from concourse import bacc, mybir, bass_utils

F32 = mybir.dt.float32
F32R = mybir.dt.float32r
BF16 = mybir.dt.bfloat16
AF = mybir.ActivationFunctionType
ALU = mybir.AluOpType

P = 128
D = 768
NH = 12
DH = 64
DFF = 3072
TB = 2048      # tokens per batch
TO = 512       # tokens owned per core
NJ = D // P    # 6 feature tiles
NT = TB // TO  # 4 token tiles per batch
NTK = TB // P  # 16 key tiles
NMLP = DFF // P  # 24
EPS = 1e-6
N_CORES = 8
VW = 66        # 64 v cols + 2 ones cols per head


def R(ap):
    return ap.bitcast(F32R)


def _build():
    nc = bacc.Bacc("TRN2", target_bir_lowering=False, debug=False,
                   num_devices=N_CORES)

    x_fm = nc.dram_tensor("x_fm", [D, TB], BF16, kind="ExternalInput").ap()
    wqkv = nc.dram_tensor("wqkv", [D, 3 * D], BF16, kind="ExternalInput").ap()
    bqk = nc.dram_tensor("bqk", [P, 12], F32, kind="ExternalInput").ap()
    wproj = nc.dram_tensor("wproj", [D, D], BF16, kind="ExternalInput").ap()
    bproj = nc.dram_tensor("bproj", [P, NJ], F32, kind="ExternalInput").ap()
    wfc1 = nc.dram_tensor("wfc1", [D, DFF], BF16, kind="ExternalInput").ap()
    bfc1 = nc.dram_tensor("bfc1", [P, NMLP], F32, kind="ExternalInput").ap()
    wfc2 = nc.dram_tensor("wfc2", [DFF, D], BF16, kind="ExternalInput").ap()
    bfc2 = nc.dram_tensor("bfc2", [P, NJ], F32, kind="ExternalInput").ap()
    selg = nc.dram_tensor("selg", [VW, 24 * NH], F32,
                          kind="ExternalInput").ap()
    selbc = nc.dram_tensor("selbc", [2 * NH, 64 * NH], F32,
                           kind="ExternalInput").ap()
    out_fm = nc.dram_tensor("out_fm", [D, TO], F32, kind="ExternalOutput").ap()

    with nc.allow_low_precision(reason="bf16 matmul operands are intentional"), \
            tile.TileContext(nc) as tc:
        _emit(tc, nc, x_fm, wqkv, bqk, wproj, bproj, wfc1, bfc1,
              wfc2, bfc2, selg, selbc, out_fm)
    nc.compile()
    return nc


def _emit(tc, nc, x_fm, wqkv, bqk, wproj_d, bproj_d, wfc1_d, bfc1_d,
          wfc2_d, bfc2_d, selg_d, selbc_d, out_fm):
    ctx_pools = []

    # Pool allocation order is dictated by LIFO release (per side); DMA
    # emission order below is independent and prioritizes x.
    cons_pool = tc.alloc_tile_pool(name="cons", bufs=1)
    ctx_pools.append(cons_pool)
    stats = tc.alloc_tile_pool(name="stats", bufs=1)
    ctx_pools.append(stats)
    x0_pool = tc.alloc_tile_pool(name="x0pool", bufs=1)
    persist = tc.alloc_tile_pool(name="persist", bufs=1)
    x_pool = tc.alloc_tile_pool(name="xpool", bufs=1)
    xn_pool = tc.alloc_tile_pool(name="xnpool", bufs=1)
    wv_pool = tc.alloc_tile_pool(name="wv", bufs=1)
    wkq_pool = tc.alloc_tile_pool(name="wkq", bufs=1)
    v_pool = tc.alloc_tile_pool(name="vpool", bufs=1, side="right")

    # ---------------- x DMAs first: nothing can start without them --------
    xt0 = [x0_pool.tile([P, TO], BF16, tag=f"x0_{j}", name=f"x0_{j}")
           for j in range(NJ)]
    xtr = [[x_pool.tile([P, TO], BF16, tag=f"x{nt}_{j}", name=f"x{nt}_{j}")
            for j in range(NJ)] for nt in range(1, NT)]
    xt = [xt0] + xtr
    for nt in range(NT):
        for j in range(NJ):
            nc.sync.dma_start(
                out=xt[nt][j],
                in_=x_fm[j * P:(j + 1) * P, nt * TO:(nt + 1) * TO])

    # qkv weights next (needed ~20us in)
    wkq = []
    for j in range(NJ):
        t = wkq_pool.tile([P, 2 * D], BF16, tag=f"wkq{j}", name=f"wkq{j}")
        nc.sync.dma_start(out=t, in_=wqkv[j * P:(j + 1) * P, 0:2 * D])
        wkq.append(t)
    wv = []
    for j in range(NJ):
        t = wv_pool.tile([P, D], BF16, tag=f"wv{j}", name=f"wv{j}")
        nc.sync.dma_start(out=t, in_=wqkv[j * P:(j + 1) * P, 2 * D:3 * D])
        wv.append(t)

    ones2 = cons_pool.tile([P, 2], F32)
    nc.vector.memset(ones2, 1.0)
    ones2b = cons_pool.tile([P, 2], BF16)
    nc.vector.memset(ones2b, 1.0)
    half2 = cons_pool.tile([2, P], F32)
    nc.vector.memset(half2, 0.5)
    eps2 = cons_pool.tile([2, 1], F32)
    nc.vector.memset(eps2, EPS)

    bqk_sb = cons_pool.tile([P, 12], F32)
    nc.sync.dma_start(out=bqk_sb, in_=bqk)
    bproj_sb = cons_pool.tile([P, NJ], F32)
    nc.sync.dma_start(out=bproj_sb, in_=bproj_d)
    bfc1_sb = cons_pool.tile([P, NMLP], F32)
    nc.sync.dma_start(out=bfc1_sb, in_=bfc1_d)
    bfc2_sb = cons_pool.tile([P, NJ], F32)
    nc.sync.dma_start(out=bfc2_sb, in_=bfc2_d)
    selg_ld = cons_pool.tile([VW, 24 * NH], F32)
    nc.sync.dma_start(out=selg_ld, in_=selg_d)
    selg_sb = cons_pool.tile([VW, 24 * NH], F32)
    nc.vector.tensor_copy(out=R(selg_sb), in_=selg_ld)
    selbc_ld = cons_pool.tile([2 * NH, 64 * NH], F32)
    nc.sync.dma_start(out=selbc_ld, in_=selbc_d)
    selbc_sb = cons_pool.tile([2 * NH, 64 * NH], F32)
    nc.vector.tensor_copy(out=R(selbc_sb), in_=selbc_ld)

    # k/q bf16; live until proj.
    k_sb = [persist.tile([P, TB], BF16, tag=f"k{j}", name=f"k{j}")
            for j in range(NJ)]
    q_sb = [persist.tile([P, TO], BF16, tag=f"q{j}", name=f"q{j}")
            for j in range(NJ)]

    v_sb = [v_pool.tile([P, NH * VW], BF16, tag=f"v{t}", name=f"v{t}")
            for t in range(NTK)]

    xn_all = [xn_pool.tile([P, TB], BF16, tag=f"xn{j}", name=f"xn{j}")
              for j in range(NJ)]

    # ---------------- Phase 1: LN1 stats (batched rsqrt chain) -----------
    # Per-nt: squares + M=2 stats matmuls as the x tiles land. One Ln+Exp
    # pair for all 2048 tokens (exactly 2 act-table loads). Then per-nt
    # broadcast + apply, interleaved with Q/K matmul emission so the PE
    # pipeline stays hot through the DVE-bound apply stream.
    sum_sb = stats.tile([2, TB], F32, tag="sum_sb", name="sum_sb")
    sq_sb = stats.tile([2, TB], F32, tag="sq_sb", name="sq_sb")
    with (
        tc.tile_pool(name="sqpool", bufs=2) as sq_pool,
        tc.tile_pool(name="lnps", bufs=2, space="PSUM") as ln_ps,
    ):
        for nt in range(NT):
            xsq = []
            for j in range(NJ):
                t = sq_pool.tile([P, TO], BF16, tag=f"xsq{j}", name="xsqt")
                nc.scalar.activation(out=t, in_=xt[nt][j], func=AF.Square)
                xsq.append(t)
            ps_sum = ln_ps.tile([2, TO], F32, tag="lnsum", name="ps_sum")
            ps_sq = ln_ps.tile([2, TO], F32, tag="lnsq", name="ps_sq")
            for j in range(NJ):
                nc.tensor.matmul(ps_sum[:], lhsT=ones2b, rhs=xt[nt][j],
                                 start=(j == 0), stop=(j == NJ - 1))
            for j in range(NJ):
                nc.tensor.matmul(ps_sq[:], lhsT=ones2b, rhs=xsq[j],
                                 start=(j == 0), stop=(j == NJ - 1))
            sl = slice(nt * TO, (nt + 1) * TO)
            nc.vector.tensor_copy(out=R(sum_sb[:, sl]), in_=ps_sum)
            nc.vector.tensor_copy(out=sq_sb[:, sl], in_=ps_sq)
    # var*D^2 = D*sumsq - sum^2 ; rs = exp(-0.5*ln(varD2/D^2 + eps)).
    # sq_sb/sum_sb are reused for intermediates to keep the [2,2048] fp32
    # column footprint at 3 tiles (8KB/partition each).
    rs = stats.tile([2, TB], F32, tag="rs", name="rs")
    nc.vector.scalar_tensor_tensor(out=R(rs), in0=sum_sb, scalar=-1.0,
                                   in1=sum_sb, op0=ALU.mult, op1=ALU.mult)
    nc.vector.scalar_tensor_tensor(out=sq_sb, in0=sq_sb, scalar=float(D),
                                   in1=rs, op0=ALU.mult, op1=ALU.add)
    nc.scalar.activation(out=sq_sb, in_=sq_sb, func=AF.Ln, bias=eps2,
                         scale=1.0 / (D * D))
    nc.scalar.activation(out=R(rs), in_=sq_sb, func=AF.Exp, scale=-0.5)
    nc.vector.scalar_tensor_tensor(out=R(sum_sb), in0=sum_sb,
                                   scalar=-1.0 / D,
                                   in1=rs, op0=ALU.mult, op1=ALU.mult)
    cc = sum_sb

    # ---------------- Phase 1b/2: LN1 apply interleaved with Q/K ----------
    with (
        tc.tile_pool(name="bcps", bufs=2, space="PSUM") as bc_ps,
        tc.tile_pool(name="tmppool", bufs=1) as tmp_pool,
        tc.tile_pool(name="mmps", bufs=4, space="PSUM") as mm_ps,
    ):
        for nt in range(NT):
            sl = slice(nt * TO, (nt + 1) * TO)
            ps_a = bc_ps.tile([P, TO], F32, tag="bca", name="ps_a")
            nc.tensor.matmul(ps_a[:], lhsT=R(half2), rhs=R(rs[:, sl]),
                             start=True, stop=True)
            ps_c = bc_ps.tile([P, TO], F32, tag="bcc", name="ps_c")
            nc.tensor.matmul(ps_c[:], lhsT=R(half2), rhs=R(cc[:, sl]),
                             start=True, stop=True)
            for j in range(NJ):
                tmp = tmp_pool.tile([P, TO], F32, tag=f"tmp{j}", name="xnt")
                nc.vector.tensor_mul(out=tmp, in0=xt[nt][j], in1=ps_a)
                nc.vector.tensor_add(out=xn_all[j][:, sl], in0=tmp, in1=ps_c)
            if nt == 0:
                # Q for own tokens (xn nt0 only)
                for m in range(NJ):
                    pt = mm_ps.tile([P, TO], F32, tag="mm", name="mmq")
                    for j in range(NJ):
                        nc.tensor.matmul(pt[:],
                                         lhsT=wkq[j][:, m * P:(m + 1) * P],
                                         rhs=xn_all[j][:, 0:TO],
                                         start=(j == 0), stop=(j == NJ - 1))
                    nc.vector.tensor_scalar_add(q_sb[m], pt,
                                                bqk_sb[:, m:m + 1])
            # K for this nt's tokens
            for m in range(NJ):
                pt = mm_ps.tile([P, TO], F32, tag="mm", name="mmk")
                for j in range(NJ):
                    nc.tensor.matmul(
                        pt[:], lhsT=wkq[j][:, D + m * P:D + (m + 1) * P],
                        rhs=xn_all[j][:, sl],
                        start=(j == 0), stop=(j == NJ - 1))
                nc.vector.tensor_scalar_add(
                    k_sb[m][:, sl], pt, bqk_sb[:, 6 + m:7 + m])
    wkq_pool.release()

    # ---------------- Phase 3: V (token-major with ones columns) ----------
    with (
        tc.tile_pool(name="vmm", bufs=4, space="PSUM") as v_ps,
    ):
        for mt in range(NTK):
            vt = v_sb[mt]
            nc.vector.memset(
                vt.rearrange("p (h w) -> p h w", w=VW)[:, :, 64:66], 1.0)
            pt5 = v_ps.tile([P, 512], F32, tag="v5", name="v5")
            pt2 = v_ps.tile([P, 256], F32, tag="v2", name="v2")
            for j in range(NJ):
                lhs = xn_all[j][:, mt * P:(mt + 1) * P]
                nc.tensor.matmul(pt5[:], lhsT=lhs, rhs=wv[j][:, 0:512],
                                 start=(j == 0), stop=(j == NJ - 1))
            for j in range(NJ):
                lhs = xn_all[j][:, mt * P:(mt + 1) * P]
                nc.tensor.matmul(pt2[:], lhsT=lhs, rhs=wv[j][:, 512:768],
                                 start=(j == 0), stop=(j == NJ - 1))
            v3 = vt.rearrange("p (h w) -> p h w", w=VW)
            nc.vector.tensor_copy(
                out=v3[:, 0:8, 0:64],
                in_=pt5.rearrange("p (h w) -> p h w", w=64))
            nc.vector.tensor_copy(
                out=v3[:, 8:12, 0:64],
                in_=pt2.rearrange("p (h w) -> p h w", w=64))
    wv_pool.release()
    xn_pool.release()
    x_pool.release()

    # ---------------- Phase 4: attention ----------------
    attn_pool = tc.alloc_tile_pool(name="attnpool", bufs=1)
    attn_fm = [attn_pool.tile([P, TO], BF16, tag=f"at{j}", name=f"at{j}")
               for j in range(NJ)]
    av_sb = [attn_pool.tile([VW, TO], F32, tag=f"av{h}", name=f"av{h}")
             for h in range(NH)]
    den24 = attn_pool.tile([2 * NH, TO], F32, tag="den", name="den")
    rinv24 = attn_pool.tile([2 * NH, TO], F32, tag="rinv", name="rinv")
    wp_pool = tc.alloc_tile_pool(name="wproj", bufs=1)
    wp = []
    for j in range(NJ):
        t = wp_pool.tile([P, D], BF16, tag=f"wp{j}", name=f"wp{j}")
        nc.sync.dma_start(out=t, in_=wproj_d[j * P:(j + 1) * P, :])
        wp.append(t)
    with (
        tc.tile_pool(name="seps", bufs=3, space="PSUM") as se_ps,
        tc.tile_pool(name="avps", bufs=1, space="PSUM") as av_ps,
        tc.tile_pool(name="sesb", bufs=6) as se_pool,
    ):
        for hp in range(NJ):
            pt_av_a = av_ps.tile([P, 512], F32, tag="ava", name="ava")
            pt_av_b = av_ps.tile([P, 512], F32, tag="avb", name="avb")
            for tk2 in range(NTK // 2):
                ps_a = se_ps.tile([P, 1024], F32, tag="se", name="psea")
                ps_b = se_ps.tile([P, 1024], F32, tag="se", name="pseb")
                for half in range(2):
                    tk = 2 * tk2 + half
                    ksl = slice(tk * P, (tk + 1) * P)
                    fsl = slice(half * 512, (half + 1) * 512)
                    nc.tensor.matmul(ps_a[:, fsl],
                                     lhsT=k_sb[hp][0:64, ksl],
                                     rhs=q_sb[hp][0:64, :],
                                     start=True, stop=True)
                    nc.tensor.matmul(ps_b[:, fsl],
                                     lhsT=k_sb[hp][64:128, ksl],
                                     rhs=q_sb[hp][64:128, :],
                                     start=True, stop=True)
                se_a = se_pool.tile([P, 1024], BF16, tag="sea", name="sea")
                se_b = se_pool.tile([P, 1024], BF16, tag="seb", name="seb")
                nc.scalar.activation(out=se_a, in_=ps_a, func=AF.Exp)
                nc.scalar.activation(out=se_b, in_=ps_b, func=AF.Exp)
                for half in range(2):
                    tk = 2 * tk2 + half
                    fsl = slice(half * 512, (half + 1) * 512)
                    first = (tk == 0)
                    last = (tk == NTK - 1)
                    nc.tensor.matmul(
                        pt_av_a[:VW, :],
                        lhsT=v_sb[tk][:, (2 * hp) * VW:(2 * hp + 1) * VW],
                        rhs=se_a[:, fsl], start=first, stop=last)
                    nc.tensor.matmul(
                        pt_av_b[:VW, :],
                        lhsT=v_sb[tk][:, (2 * hp + 1) * VW:(2 * hp + 2) * VW],
                        rhs=se_b[:, fsl], start=first, stop=last)
            for head, pt_av in ((0, pt_av_a), (1, pt_av_b)):
                # Evacuate numerator + denominator rows now (DVE is idle
                # during the ACT-bound exp stream); reciprocal deferred to
                # one batched exp(-ln) on ACT after the last exp.
                h = 2 * hp + head
                nc.vector.tensor_copy(out=R(av_sb[h]), in_=pt_av[0:VW, :])
    # Gather the 12 denominator pairs into [24, 512] via selection matmuls
    # (partition offsets must be 32-aligned, so no direct per-head copies),
    # then one batched reciprocal: 1/x = exp(-ln(x)).
    with (
        tc.tile_pool(name="divps", bufs=4, space="PSUM") as div_ps,
    ):
        ps_den = div_ps.tile([2 * NH, TO], F32, tag="den", name="psden")
        for h in range(NH):
            nc.tensor.matmul(ps_den[:],
                             lhsT=R(selg_sb[:, 24 * h:24 * (h + 1)]),
                             rhs=R(av_sb[h]),
                             start=(h == 0), stop=(h == NH - 1))
        nc.vector.tensor_copy(out=den24, in_=ps_den)
        nc.scalar.activation(out=den24, in_=den24, func=AF.Ln)
        nc.scalar.activation(out=R(rinv24), in_=den24, func=AF.Exp,
                             scale=-1.0)
        # Broadcast 1/den over the 64 v rows, multiply straight from PSUM.
        for h in range(NH):
            ps_bc = div_ps.tile([64, 512], F32, tag="bc", name="psbc")
            nc.tensor.matmul(ps_bc[:],
                             lhsT=R(selbc_sb[:, h * 64:(h + 1) * 64]),
                             rhs=R(rinv24), start=True, stop=True)
            hp, head = divmod(h, 2)
            nc.vector.tensor_mul(
                out=attn_fm[hp][head * 64:(head + 1) * 64, :],
                in0=av_sb[h][0:64, :], in1=ps_bc)
    v_pool.release()

    # ---------------- Phase 5: proj + residual -> x2 (fp32) ----------------
    x2_pool = tc.alloc_tile_pool(name="x2pool", bufs=1, side="right")
    ctx_pools.append(x2_pool)
    x2_sb = [x2_pool.tile([P, TO], F32, tag=f"x2{j}", name=f"x2{j}")
             for j in range(NJ)]
    wfc1_pool = tc.alloc_tile_pool(name="wfc1", bufs=1, side="right")
    ctx_pools.append(wfc1_pool)
    wf1 = []
    for j in range(NJ):
        t = wfc1_pool.tile([P, DFF], BF16, tag=f"wf1{j}", name=f"wf1{j}")
        nc.sync.dma_start(out=t, in_=wfc1_d[j * P:(j + 1) * P, :])
        wf1.append(t)
    with (
        tc.tile_pool(name="mmps2", bufs=3, space="PSUM") as mm_ps2,
    ):
        for m in range(NJ):
            pt = mm_ps2.tile([P, TO], F32, tag="mm", name="mmproj")
            for j in range(NJ):
                nc.tensor.matmul(pt[:], lhsT=wp[j][:, m * P:(m + 1) * P],
                                 rhs=attn_fm[j],
                                 start=(j == 0), stop=(j == NJ - 1))
            nc.vector.scalar_tensor_tensor(
                out=R(x2_sb[m]), in0=pt, scalar=bproj_sb[:, m:m + 1],
                in1=xt0[m], op0=ALU.add, op1=ALU.add)
    wp_pool.release()
    attn_pool.release()
    persist.release()
    x0_pool.release()

    # ---------------- Phase 6: LN2 -> h (bf16) ----------------
    h_pool = tc.alloc_tile_pool(name="hpool", bufs=1, side="right")
    ctx_pools.append(h_pool)
    h_sb = [h_pool.tile([P, TO], BF16, tag=f"h{j}", name=f"h{j}")
            for j in range(NJ)]
    with (
        tc.tile_pool(name="sqpool2", bufs=2) as sq_pool2,
        tc.tile_pool(name="lnps2", bufs=1, space="PSUM") as ln_ps2,
        tc.tile_pool(name="bcps3", bufs=1, space="PSUM") as bc_ps3,
    ):
        xsq = []
        for j in range(NJ):
            t = sq_pool2.tile([P, TO], F32, tag=f"x2sq{j}", name="x2sqt")
            nc.scalar.activation(out=R(t), in_=x2_sb[j], func=AF.Square)
            xsq.append(t)
        ps_sum = ln_ps2.tile([2, TO], F32, tag="lnsum2", name="ps_sum2")
        ps_sq = ln_ps2.tile([2, TO], F32, tag="lnsq2", name="ps_sq2")
        for j in range(NJ):
            nc.tensor.matmul(ps_sum[:], lhsT=R(ones2), rhs=R(x2_sb[j]),
                             start=(j == 0), stop=(j == NJ - 1))
        for j in range(NJ):
            nc.tensor.matmul(ps_sq[:], lhsT=R(ones2), rhs=R(xsq[j]),
                             start=(j == 0), stop=(j == NJ - 1))
        sum2 = stats.tile([2, TO], F32, tag="sum2", name="sum2")
        sq2 = stats.tile([2, TO], F32, tag="sq2", name="sq2")
        nc.vector.tensor_copy(out=sum2, in_=ps_sum)
        nc.vector.tensor_copy(out=sq2, in_=ps_sq)
        t2 = stats.tile([2, TO], F32, tag="t2", name="t2")
        nc.vector.scalar_tensor_tensor(out=t2, in0=sum2, scalar=-1.0,
                                       in1=sum2, op0=ALU.mult, op1=ALU.mult)
        nc.vector.scalar_tensor_tensor(out=t2, in0=sq2, scalar=float(D),
                                       in1=t2, op0=ALU.mult, op1=ALU.add)
        nc.scalar.activation(out=t2, in_=t2, func=AF.Ln, bias=eps2,
                             scale=1.0 / (D * D))
        rs2 = stats.tile([2, TO], F32, tag="rs2", name="rs2")
        nc.scalar.activation(out=R(rs2), in_=t2, func=AF.Exp, scale=-0.5)
        cc2 = stats.tile([2, TO], F32, tag="cc2", name="cc2")
        nc.vector.scalar_tensor_tensor(out=R(cc2), in0=sum2, scalar=-1.0 / D,
                                       in1=rs2, op0=ALU.mult, op1=ALU.mult)
        ps_a = bc_ps3.tile([P, TO], F32, tag="bca2", name="ps_a2")
        nc.tensor.matmul(ps_a[:], lhsT=R(half2), rhs=R(rs2),
                         start=True, stop=True)
        ps_c = bc_ps3.tile([P, TO], F32, tag="bcc2", name="ps_c2")
        nc.tensor.matmul(ps_c[:], lhsT=R(half2), rhs=R(cc2),
                         start=True, stop=True)
        for j in range(NJ):
            tmp = sq_pool2.tile([P, TO], F32, tag=f"tmp2{j}", name="h2t")
            nc.vector.tensor_mul(out=tmp, in0=x2_sb[j], in1=ps_a)
            nc.vector.tensor_add(out=h_sb[j], in0=tmp, in1=ps_c)

    # ---------------- Phase 7: fc1 + gelu -> h1 (bf16) ----------------
    h1_pool = tc.alloc_tile_pool(name="h1", bufs=1, side="right")
    ctx_pools.append(h1_pool)
    h1_sb = [h1_pool.tile([P, TO], BF16, tag=f"h1{m}", name=f"h1{m}")
             for m in range(NMLP)]
    with (
        tc.tile_pool(name="mmps3", bufs=4, space="PSUM") as mm_ps3,
    ):
        for m in range(NMLP):
            pt = mm_ps3.tile([P, TO], F32, tag="mm", name="mmfc1")
            for j in range(NJ):
                nc.tensor.matmul(pt[:], lhsT=wf1[j][:, m * P:(m + 1) * P],
                                 rhs=h_sb[j],
                                 start=(j == 0), stop=(j == NJ - 1))
            nc.scalar.activation(out=h1_sb[m], in_=pt, func=AF.Gelu,
                                 bias=bfc1_sb[:, m:m + 1])

    # ---------------- Phase 8: fc2 + residual + store ----------------
    with (
        tc.tile_pool(name="wfc2", bufs=6) as wfc2_pool,
        tc.tile_pool(name="fc2ps", bufs=1, space="PSUM") as fc2_ps,
        tc.tile_pool(name="outsb", bufs=2) as out_pool,
    ):
        pts = [fc2_ps.tile([P, TO], F32, tag=f"fc2_{m}", name=f"fc2_{m}")
               for m in range(NJ)]
        for j in range(NMLP):
            wt = wfc2_pool.tile([P, D], BF16, tag="wf2", name="wf2")
            nc.sync.dma_start(out=wt, in_=wfc2_d[j * P:(j + 1) * P, :])
            for m in range(NJ):
                nc.tensor.matmul(pts[m][:], lhsT=wt[:, m * P:(m + 1) * P],
                                 rhs=h1_sb[j],
                                 start=(j == 0), stop=(j == NMLP - 1))
        for m in range(NJ):
            ot = out_pool.tile([P, TO], F32, tag="out", name="ot")
            nc.vector.scalar_tensor_tensor(
                out=ot, in0=pts[m], scalar=bfc2_sb[:, m:m + 1],
                in1=x2_sb[m], op0=ALU.add, op1=ALU.add)
            nc.sync.dma_start(out=out_fm[m * P:(m + 1) * P, :], in_=ot)

    for pool in reversed(ctx_pools):
        pool.release()


_NC_CACHE = {}


def _get_nc():
    if "nc" not in _NC_CACHE:
        _NC_CACHE["nc"] = _build()
    return _NC_CACHE["nc"]


def _host_prep(inputs):
    f32 = lambda a: np.ascontiguousarray(np.asarray(a, dtype=np.float32))
    x = f32(inputs["x"])            # [2, 2048, 768]
    W_qkv = f32(inputs["W_qkv"])    # [768, 2304]
    b_qkv = f32(inputs["b_qkv"])
    W_proj = f32(inputs["W_proj"])
    b_proj = f32(inputs["b_proj"])
    W_fc1 = f32(inputs["W_fc1"])
    b_fc1 = f32(inputs["b_fc1"])
    W_fc2 = f32(inputs["W_fc2"])
    b_fc2 = f32(inputs["b_fc2"])
    ln1_g = f32(inputs["ln1_g"])
    ln1_b = f32(inputs["ln1_b"])
    ln2_g = f32(inputs["ln2_g"])
    ln2_b = f32(inputs["ln2_b"])

    scale = DH ** -0.5
    wqkv_eff = W_qkv * ln1_g[:, None]
    bqkv_eff = ln1_b @ W_qkv + b_qkv
    wqkv_eff[:, :D] *= scale
    bqkv_eff_q = bqkv_eff[:D] * scale
    bqk = np.concatenate([bqkv_eff_q, bqkv_eff[D:2 * D]]).astype(np.float32)
    bv = bqkv_eff[2 * D:]
    bproj_eff = (b_proj + bv @ W_proj).astype(np.float32)
    wfc1_eff = (W_fc1 * ln2_g[:, None]).astype(np.float32)
    bfc1_eff = (ln2_b @ W_fc1 + b_fc1).astype(np.float32)

    # Selection constants for softmax-denominator gather / broadcast.
    selg = np.zeros((VW, 24 * NH), dtype=np.float32)
    selbc = np.zeros((2 * NH, 64 * NH), dtype=np.float32)
    for h in range(NH):
        selg[64:66, 24 * h + 2 * h:24 * h + 2 * h + 2] = np.eye(2)
        selbc[2 * h:2 * h + 2, h * 64:(h + 1) * 64] = 0.5

    bf = lambda a: np.ascontiguousarray(a.astype(ml_dtypes.bfloat16))
    pack = lambda b: np.ascontiguousarray(
        b.reshape(-1, P).T.astype(np.float32))
    shared = {
        "selg": selg,
        "selbc": selbc,
        "wqkv": bf(wqkv_eff),
        "bqk": pack(bqk),
        "wproj": bf(W_proj),
        "bproj": pack(bproj_eff),
        "wfc1": bf(wfc1_eff),
        "bfc1": pack(bfc1_eff),
        "wfc2": bf(W_fc2),
        "bfc2": pack(b_fc2),
    }
    in_maps = []
    for c in range(N_CORES):
        b, q = divmod(c, 4)
        xb = np.roll(x[b], -TO * q, axis=0)  # own tokens at rows 0:TO
        m = dict(shared)
        m["x_fm"] = bf(xb.T)
        in_maps.append(m)
    return in_maps


def _run(inputs, trace=False):
    nc = _get_nc()
    in_maps = _host_prep(inputs)
    res = bass_utils.run_bass_kernel_spmd(nc, in_maps, list(range(N_CORES)),
                                          trace=trace)
    B = 2
    out = np.empty((B, TB, D), dtype=np.float32)
    for c in range(N_CORES):
        b, q = divmod(c, 4)
        out[b, TO * q:TO * (q + 1), :] = res.results[c]["out_fm"].T
    return out, res


def kernel(**inputs):
    out, _ = _run(inputs, trace=False)
    return out


if __name__ == "__main__":
    print("building...")
    _get_nc()
    print("built ok")


# revision 51
# speedup vs baseline: 1.3349x; 1.0241x over previous
"""Trainium2 Bass kernel for a dense transformer block (pre-LN, MHA + GELU MLP).

Sharding: 8 cores = 2 batches x 4 sequence-quarters. Each core recomputes
LN1 + K/V for its full batch (zero cross-core communication), and computes
Q/attention/proj/MLP for its own 512 tokens only.

Device works feature-major ([feature, token]); the host pre-transposes x and
post-transposes the output. LN gains/biases are folded into the following
matmul weights on the host; the qk scale (1/8) is folded into W_q; the v bias
is folded into b_proj.

Numerics: matmul operands are bf16 (fp32 PSUM accumulation); the residual
stream (x2, out), layernorm statistics, and softmax denominators stay fp32.
The attention residual uses the bf16 x tiles (error well inside tolerance).

Schedule (v3): the attention exp stream is ACT-bound (~105us at 1 elem/
cycle/lane) while QK+AV leave the PE half idle, and the PE drops to a 1.2GHz
p-state whenever it stalls. So all K (head-pairs 1-5) and V matmuls are
emitted as PE backfill *inside* the attention windows: V and K(1) during the
head-pair-0 window (merged with the LN1 apply loop), K(m) during window m-1.
AV consumption lags QK/exp by one step. PSUM budget: 4 banks score
ping/pong + 2 AV accumulators + 2 misc = 8.

Activation-table hygiene: the act-table pass greedily maps Ln->natural_log,
Exp->exp_and_others, so each Ln/Exp pair costs two ~1.3us table loads. LN
rsqrt is therefore a single Rsqrt (reciprocal_sqrt_and_small set, preloaded
by a dummy at t=0), the softmax reciprocal is rsqrt(x)^2 (same set), and Exp
is preloaded by a dummy before the attention stream. Square is filler in
every set and never forces a load.
"""
import sys

sys.path.insert(0, "/opt/trn_rl_repo")

import numpy as np
import ml_dtypes

import concourse.bass as bass  # noqa: F401
import concourse.tile as tile
from concourse import bacc, mybir, bass_utils

F32 = mybir.dt.float32
F32R = mybir.dt.float32r
BF16 = mybir.dt.bfloat16
AF = mybir.ActivationFunctionType
ALU = mybir.AluOpType

P = 128
D = 768
NH = 12
DH = 64
DFF = 3072
TB = 2048      # tokens per batch
TO = 512       # tokens owned per core
NJ = D // P    # 6 feature tiles
NT = TB // TO  # 4 token tiles per batch
NTK = TB // P  # 16 key tiles
NMLP = DFF // P  # 24
EPS = 1e-6
N_CORES = 8
VW = 66        # 64 v cols + 2 ones cols per head


def R(ap):
    return ap.bitcast(F32R)


def _build():
    nc = bacc.Bacc("TRN2", target_bir_lowering=False, debug=False,
                   num_devices=N_CORES)

    x_fm = nc.dram_tensor("x_fm", [D, TB], BF16, kind="ExternalInput").ap()
    wqkv = nc.dram_tensor("wqkv", [D, 3 * D], BF16, kind="ExternalInput").ap()
    bqk = nc.dram_tensor("bqk", [P, 12], F32, kind="ExternalInput").ap()
    wproj = nc.dram_tensor("wproj", [D, D], BF16, kind="ExternalInput").ap()
    bproj = nc.dram_tensor("bproj", [P, NJ], F32, kind="ExternalInput").ap()
    wfc1 = nc.dram_tensor("wfc1", [D, DFF], BF16, kind="ExternalInput").ap()
    bfc1 = nc.dram_tensor("bfc1", [P, NMLP], F32, kind="ExternalInput").ap()
    wfc2 = nc.dram_tensor("wfc2", [DFF, D], BF16, kind="ExternalInput").ap()
    bfc2 = nc.dram_tensor("bfc2", [P, NJ], F32, kind="ExternalInput").ap()
    selg = nc.dram_tensor("selg", [VW, 24 * NH], BF16,
                          kind="ExternalInput").ap()
    selbc = nc.dram_tensor("selbc", [2 * NH, 64 * NH], F32,
                           kind="ExternalInput").ap()
    out_fm = nc.dram_tensor("out_fm", [D, TO], F32, kind="ExternalOutput").ap()

    with nc.allow_low_precision(reason="bf16 matmul operands are intentional"), \
            tile.TileContext(nc) as tc:
        _emit(tc, nc, x_fm, wqkv, bqk, wproj, bproj, wfc1, bfc1,
              wfc2, bfc2, selg, selbc, out_fm)
    nc.compile()
    return nc


def _emit(tc, nc, x_fm, wqkv, bqk, wproj_d, bproj_d, wfc1_d, bfc1_d,
          wfc2_d, bfc2_d, selg_d, selbc_d, out_fm):
    ctx_pools = []

    # Left-stack alloc order is dictated by LIFO release:
    # cons, x0, persist, attn, wp, wkq, wv, xn, x, stats1.
    # Right stack: wfc1 (ctx), v (released after div).
    cons_pool = tc.alloc_tile_pool(name="cons", bufs=1)
    ctx_pools.append(cons_pool)
    x0_pool = tc.alloc_tile_pool(name="x0pool", bufs=1)
    persist = tc.alloc_tile_pool(name="persist", bufs=1)
    attn_pool = tc.alloc_tile_pool(name="attnpool", bufs=1)
    wp_pool = tc.alloc_tile_pool(name="wproj", bufs=1)
    wkq_pool = tc.alloc_tile_pool(name="wkq", bufs=1)
    xn0_pool = tc.alloc_tile_pool(name="xn0pool", bufs=1)
    x_pool = tc.alloc_tile_pool(name="xpool", bufs=1)
    se_pool = tc.alloc_tile_pool(name="sesb", bufs=2)
    wv_pool = tc.alloc_tile_pool(name="wv", bufs=1)
    ab_pool = tc.alloc_tile_pool(name="abpool", bufs=1)
    stats1 = tc.alloc_tile_pool(name="stats1", bufs=1)
    stats_sq = tc.alloc_tile_pool(name="statsq", bufs=1)
    v_pool = tc.alloc_tile_pool(name="vpool", bufs=1, side="right")

    # ---------------- DMAs in priority order -----------------------------
    xt0 = [x0_pool.tile([P, TO], BF16, tag=f"x0_{j}", name=f"x0_{j}")
           for j in range(NJ)]
    xtr = [[x_pool.tile([P, TO], BF16, tag=f"x{nt}_{j}", name=f"x{nt}_{j}")
            for j in range(NJ)] for nt in range(1, NT)]
    xt = [xt0] + xtr
    for nt in range(NT):
        for j in range(NJ):
            nc.sync.dma_start(
                out=xt[nt][j],
                in_=x_fm[j * P:(j + 1) * P, nt * TO:(nt + 1) * TO])
    wkq = []
    for j in range(NJ):
        t = wkq_pool.tile([P, 2 * D], BF16, tag=f"wkq{j}", name=f"wkq{j}")
        nc.sync.dma_start(out=t, in_=wqkv[j * P:(j + 1) * P, 0:2 * D])
        wkq.append(t)
    wv = []
    for j in range(NJ):
        t = wv_pool.tile([P, D], BF16, tag=f"wv{j}", name=f"wv{j}")
        nc.sync.dma_start(out=t, in_=wqkv[j * P:(j + 1) * P, 2 * D:3 * D])
        wv.append(t)
    wp = []
    for j in range(NJ):
        t = wp_pool.tile([P, D], BF16, tag=f"wp{j}", name=f"wp{j}")
        nc.sync.dma_start(out=t, in_=wproj_d[j * P:(j + 1) * P, :])
        wp.append(t)

    ones2b = cons_pool.tile([P, 2], BF16)
    nc.vector.memset(ones2b, 1.0)
    ones2 = cons_pool.tile([P, 2], F32)
    nc.vector.memset(ones2, 1.0)
    half2 = cons_pool.tile([2, P], F32)
    nc.vector.memset(half2, 0.5)
    eps2 = cons_pool.tile([2, 1], F32)
    nc.vector.memset(eps2, EPS)
    dummy = cons_pool.tile([2, 1], F32)
    # Preload the natural_log act table during the x DMA wait (Rsqrt is
    # blocked by bass for accuracy, so LN uses exp(-0.5*ln(var))).
    nc.scalar.activation(out=dummy, in_=eps2, func=AF.Ln)

    bqk_sb = cons_pool.tile([P, 12], F32)
    nc.sync.dma_start(out=bqk_sb, in_=bqk)
    bproj_sb = cons_pool.tile([P, NJ], F32)
    nc.sync.dma_start(out=bproj_sb, in_=bproj_d)
    bfc1_sb = cons_pool.tile([P, NMLP], F32)
    nc.sync.dma_start(out=bfc1_sb, in_=bfc1_d)
    bfc2_sb = cons_pool.tile([P, NJ], F32)
    nc.sync.dma_start(out=bfc2_sb, in_=bfc2_d)
    selg_sb = cons_pool.tile([VW, 24 * NH], BF16)
    nc.sync.dma_start(out=selg_sb, in_=selg_d)
    selbc_ld = cons_pool.tile([2 * NH, 64 * NH], F32)
    nc.sync.dma_start(out=selbc_ld, in_=selbc_d)
    selbc_sb = cons_pool.tile([2 * NH, 64 * NH], F32)
    nc.vector.tensor_copy(out=R(selbc_sb), in_=selbc_ld)

    k_sb = [persist.tile([P, TB], BF16, tag=f"k{j}", name=f"k{j}")
            for j in range(NJ)]
    q_sb = [persist.tile([P, TO], BF16, tag=f"q{j}", name=f"q{j}")
            for j in range(NJ)]
    attn_fm = [attn_pool.tile([P, TO], BF16, tag=f"at{j}", name=f"at{j}")
               for j in range(NJ)]
    av_sb = [attn_pool.tile([VW, TO], BF16, tag=f"av{h}", name=f"av{h}")
             for h in range(NH)]
    den24 = attn_pool.tile([2 * NH, TO], F32, tag="den", name="den")
    rinv24 = attn_pool.tile([2 * NH, TO], F32, tag="rinv", name="rinv")
    v_sb = [v_pool.tile([P, NH * VW], BF16, tag=f"v{t}", name=f"v{t}")
            for t in range(NTK)]
    # LN1 output: fresh tiles for nt0 (the raw x0 is the proj residual);
    # written in place over the x tiles for nt>=1.
    xn0 = [xn0_pool.tile([P, TO], BF16, tag=f"xn0_{j}", name=f"xn0_{j}")
           for j in range(NJ)]
    xn_t = [xn0] + xtr

    def xn_cols(j, mt):
        nt, sub = divmod(mt, 4)
        return xn_t[nt][j][:, sub * P:(sub + 1) * P]

    # ---------------- Phase 1a: LN1 stats as x tiles land ------------------
    sum_sb = stats1.tile([2, TB], F32, tag="sum_sb", name="sum_sb")
    rs = stats1.tile([2, TB], F32, tag="rs", name="rs")
    sq_sb = stats_sq.tile([2, TB], F32, tag="sq_sb", name="sq_sb")
    with (
        tc.tile_pool(name="sqpool", bufs=1) as sq_pool,
        tc.tile_pool(name="lnps", bufs=2, space="PSUM") as ln_ps,
    ):
        for nt in range(NT):
            xsq = []
            for j in range(NJ):
                t = sq_pool.tile([P, TO], BF16, tag=f"xsq{j}", name="xsqt")
                nc.scalar.activation(out=t, in_=xt[nt][j], func=AF.Square)
                xsq.append(t)
            ps_sum = ln_ps.tile([2, TO], F32, tag="lnsum", name="ps_sum")
            ps_sq = ln_ps.tile([2, TO], F32, tag="lnsq", name="ps_sq")
            for j in range(NJ):
                nc.tensor.matmul(ps_sum[:], lhsT=ones2b, rhs=xt[nt][j],
                                 start=(j == 0), stop=(j == NJ - 1))
            for j in range(NJ):
                nc.tensor.matmul(ps_sq[:], lhsT=ones2b, rhs=xsq[j],
                                 start=(j == 0), stop=(j == NJ - 1))
            sl = slice(nt * TO, (nt + 1) * TO)
            nc.vector.tensor_copy(out=R(sum_sb[:, sl]), in_=ps_sum)
            nc.vector.tensor_copy(out=sq_sb[:, sl], in_=ps_sq)
    # var*D^2 = D*sumsq - sum^2 ; rs = exp(-0.5*ln(varD2/D^2 + eps)). The
    # ln table is resident from the t=0 dummy; the Exp pulls in the exp set
    # that the attention stream then keeps.
    nc.vector.scalar_tensor_tensor(out=R(rs), in0=sum_sb, scalar=-1.0,
                                   in1=sum_sb, op0=ALU.mult, op1=ALU.mult)
    nc.vector.scalar_tensor_tensor(out=sq_sb, in0=sq_sb, scalar=float(D),
                                   in1=rs, op0=ALU.mult, op1=ALU.add)
    nc.scalar.activation(out=sq_sb, in_=sq_sb, func=AF.Ln, bias=eps2,
                         scale=1.0 / (D * D))
    nc.scalar.activation(out=R(rs), in_=sq_sb, func=AF.Exp, scale=-0.5)
    nc.vector.scalar_tensor_tensor(out=R(sum_sb), in0=sum_sb,
                                   scalar=-1.0 / D,
                                   in1=rs, op0=ALU.mult, op1=ALU.mult)
    cc = sum_sb
    stats_sq.release()

    # ------- Merged window: LN1 apply + Q + K0/K1 + V + attention hp0 ------
    # Per nt: broadcast a/c, apply, then this nt's K(0), V tiles, and the
    # two hp0 QK/exp steps they unlock. AV lags one step. K(1) rides along
    # so window hp1 can start clean.
    se_ps_pool = tc.tile_pool(name="seps", bufs=1, space="PSUM")
    av_ps_pool = tc.tile_pool(name="avps", bufs=1, space="PSUM")
    misc_ps_pool = tc.tile_pool(name="miscps", bufs=2, space="PSUM")
    se_ps = se_ps_pool.__enter__()
    av_ps = av_ps_pool.__enter__()
    misc_ps = misc_ps_pool.__enter__()

    av_pairs = {}
    se_tiles = {}

    def emit_K(m, nt):
        sl = slice(nt * TO, (nt + 1) * TO)
        pt = misc_ps.tile([P, TO], F32, tag="mm", name=f"mmk{m}_{nt}")
        for j in range(NJ):
            nc.tensor.matmul(pt[:], lhsT=wkq[j][:, D + m * P:D + (m + 1) * P],
                             rhs=xn_t[nt][j],
                             start=(j == 0), stop=(j == NJ - 1))
        nc.vector.tensor_scalar_add(k_sb[m][:, sl], pt, bqk_sb[:, 6 + m:7 + m])

    def emit_V(mt):
        vt = v_sb[mt]
        nc.vector.memset(
            vt.rearrange("p (h w) -> p h w", w=VW)[:, :, 64:66], 1.0)
        pt5 = misc_ps.tile([P, TO], F32, tag="mm", name=f"v5_{mt}")
        for j in range(NJ):
            nc.tensor.matmul(pt5[:], lhsT=xn_cols(j, mt), rhs=wv[j][:, 0:512],
                             start=(j == 0), stop=(j == NJ - 1))
        pt2 = misc_ps.tile([P, TO], F32, tag="mm", name=f"v2_{mt}")
        for j in range(NJ):
            nc.tensor.matmul(pt2[:, 0:256], lhsT=xn_cols(j, mt),
                             rhs=wv[j][:, 512:768],
                             start=(j == 0), stop=(j == NJ - 1))
        v3 = vt.rearrange("p (h w) -> p h w", w=VW)
        nc.vector.tensor_copy(
            out=v3[:, 0:8, 0:64],
            in_=pt5.rearrange("p (h w) -> p h w", w=64))
        nc.vector.tensor_copy(
            out=v3[:, 8:12, 0:64],
            in_=pt2[:, 0:256].rearrange("p (h w) -> p h w", w=64))

    def emit_QK(hp, s):
        ps_a = se_ps.tile([P, 1024], F32, tag="sea", name=f"psea{hp}_{s}")
        ps_b = se_ps.tile([P, 1024], F32, tag="seb", name=f"pseb{hp}_{s}")
        for half in range(2):
            tk = 2 * s + half
            ksl = slice(tk * P, (tk + 1) * P)
            fsl = slice(half * 512, (half + 1) * 512)
            nc.tensor.matmul(ps_a[:, fsl], lhsT=k_sb[hp][0:64, ksl],
                             rhs=q_sb[hp][0:64, :], start=True, stop=True)
            nc.tensor.matmul(ps_b[:, fsl], lhsT=k_sb[hp][64:128, ksl],
                             rhs=q_sb[hp][64:128, :], start=True, stop=True)
        se_a = se_pool.tile([P, 1024], BF16, tag="sea", name=f"sea{hp}_{s}")
        se_b = se_pool.tile([P, 1024], BF16, tag="seb", name=f"seb{hp}_{s}")
        nc.scalar.activation(out=se_a, in_=ps_a, func=AF.Exp)
        nc.scalar.activation(out=se_b, in_=ps_b, func=AF.Exp)
        se_tiles[(hp, s)] = (se_a, se_b)

    def emit_AV(hp, s):
        if s == 0:
            av_pairs[hp] = (
                av_ps.tile([P, 512], F32, tag="ava", name=f"ava{hp}"),
                av_ps.tile([P, 512], F32, tag="avb", name=f"avb{hp}"),
            )
        pt_av_a, pt_av_b = av_pairs[hp]
        se_a, se_b = se_tiles.pop((hp, s))
        for half in range(2):
            tk = 2 * s + half
            fsl = slice(half * 512, (half + 1) * 512)
            first = (tk == 0)
            last = (tk == NTK - 1)
            nc.tensor.matmul(
                pt_av_a[:VW, :],
                lhsT=v_sb[tk][:, (2 * hp) * VW:(2 * hp + 1) * VW],
                rhs=se_a[:, fsl], start=first, stop=last)
            nc.tensor.matmul(
                pt_av_b[:VW, :],
                lhsT=v_sb[tk][:, (2 * hp + 1) * VW:(2 * hp + 2) * VW],
                rhs=se_b[:, fsl], start=first, stop=last)

    def emit_evac(hp):
        pt_av_a, pt_av_b = av_pairs.pop(hp)
        for head, pt_av in ((0, pt_av_a), (1, pt_av_b)):
            h = 2 * hp + head
            nc.vector.tensor_copy(out=av_sb[h], in_=pt_av[0:VW, :])

    for nt in range(NT):
        sl = slice(nt * TO, (nt + 1) * TO)
        ps_a = misc_ps.tile([P, TO], F32, tag="mm", name=f"ps_a{nt}")
        nc.tensor.matmul(ps_a[:], lhsT=R(half2), rhs=R(rs[:, sl]),
                         start=True, stop=True)
        a_bf = ab_pool.tile([P, TO], BF16, tag="a", name=f"a{nt}")
        nc.vector.tensor_copy(out=a_bf, in_=ps_a)
        ps_c = misc_ps.tile([P, TO], F32, tag="mm", name=f"ps_c{nt}")
        nc.tensor.matmul(ps_c[:], lhsT=R(half2), rhs=R(cc[:, sl]),
                         start=True, stop=True)
        c_bf = ab_pool.tile([P, TO], BF16, tag="c", name=f"c{nt}")
        nc.vector.tensor_copy(out=c_bf, in_=ps_c)
        # bf16 2x-rate applies; nt>=1 writes xn in place over the x tile
        for j in range(NJ):
            tmp = ab_pool.tile([P, TO], BF16, tag=f"t{j}", name="xnt")
            nc.vector.tensor_mul(out=tmp, in0=xt[nt][j], in1=a_bf)
            nc.vector.tensor_add(out=xn_t[nt][j], in0=tmp, in1=c_bf)
        if nt >= 1:
            emit_AV(0, 2 * (nt - 1))
            emit_AV(0, 2 * (nt - 1) + 1)
        if nt == 0:
            for m in range(NJ):
                pt = misc_ps.tile([P, TO], F32, tag="mm", name=f"mmq{m}")
                for j in range(NJ):
                    nc.tensor.matmul(pt[:],
                                     lhsT=wkq[j][:, m * P:(m + 1) * P],
                                     rhs=xn0[j],
                                     start=(j == 0), stop=(j == NJ - 1))
                nc.vector.tensor_scalar_add(q_sb[m], pt,
                                            bqk_sb[:, m:m + 1])
        emit_K(0, nt)
        emit_K(1, nt)
        for mt in range(4 * nt, 4 * nt + 4):
            emit_V(mt)
        emit_QK(0, 2 * nt)
        emit_QK(0, 2 * nt + 1)
    emit_AV(0, 6)
    emit_AV(0, 7)
    emit_evac(0)
    stats1.release()
    ab_pool.release()
    wv_pool.release()

    # ---------------- Attention windows hp=1..5 ---------------------------
    for hp in range(1, NJ):
        for s in range(8):
            emit_QK(hp, s)
            if s >= 1:
                emit_AV(hp, s - 1)
            if s % 2 == 1 and hp <= 4:
                emit_K(hp + 1, (s - 1) // 2)
        emit_AV(hp, 7)
        emit_evac(hp)
    se_pool.release()
    x_pool.release()
    xn0_pool.release()
    wkq_pool.release()
    misc_ps_pool.__exit__(None, None, None)
    av_ps_pool.__exit__(None, None, None)
    se_ps_pool.__exit__(None, None, None)

    # ---------------- Softmax denominators + division --------------------
    # Gather the 12 denominator pairs into [24, 512] via selection matmuls
    # (partition offsets must be 32-aligned, so no direct per-head copies),
    # then one batched reciprocal: 1/x = rsqrt(x)^2 (table already loaded).
    with (
        tc.tile_pool(name="divps", bufs=2, space="PSUM") as div_ps,
        tc.tile_pool(name="denps", bufs=1, space="PSUM") as den_ps,
    ):
        ps_den = den_ps.tile([2 * NH, TO], F32, tag="den", name="psden")
        for h in range(NH):
            nc.tensor.matmul(ps_den[:],
                             lhsT=selg_sb[:, 24 * h:24 * (h + 1)],
                             rhs=av_sb[h],
                             start=(h == 0), stop=(h == NH - 1))
        nc.vector.tensor_copy(out=den24, in_=ps_den)
        nc.scalar.activation(out=den24, in_=den24, func=AF.Ln)
        nc.scalar.activation(out=R(rinv24), in_=den24, func=AF.Exp,
                             scale=-1.0)
        # Broadcast 1/den over the 64 v rows, multiply straight from PSUM.
        for h in range(NH):
            ps_bc = div_ps.tile([64, 512], F32, tag="bc", name="psbc")
            nc.tensor.matmul(ps_bc[:],
                             lhsT=R(selbc_sb[:, h * 64:(h + 1) * 64]),
                             rhs=R(rinv24), start=True, stop=True)
            hp, head = divmod(h, 2)
            nc.vector.tensor_mul(
                out=attn_fm[hp][head * 64:(head + 1) * 64, :],
                in0=av_sb[h][0:64, :], in1=ps_bc)
    v_pool.release()

    # ---------------- proj + residual -> x2, LN2 stats interleaved --------
    x2_pool = tc.alloc_tile_pool(name="x2pool", bufs=1, side="right")
    ctx_pools.append(x2_pool)
    x2_sb = [x2_pool.tile([P, TO], F32, tag=f"x2{j}", name=f"x2{j}")
             for j in range(NJ)]
    wfc1_pool = tc.alloc_tile_pool(name="wfc1", bufs=1, side="right")
    ctx_pools.append(wfc1_pool)
    wf1 = []
    for j in range(NJ):
        t = wfc1_pool.tile([P, DFF], BF16, tag=f"wf1{j}", name=f"wf1{j}")
        nc.sync.dma_start(out=t, in_=wfc1_d[j * P:(j + 1) * P, :])
        wf1.append(t)
    h_pool = tc.alloc_tile_pool(name="hpool", bufs=1, side="right")
    ctx_pools.append(h_pool)
    h_sb = [h_pool.tile([P, TO], BF16, tag=f"h{j}", name=f"h{j}")
            for j in range(NJ)]
    with (
        tc.tile_pool(name="mmps2", bufs=3, space="PSUM") as mm_ps2,
        tc.tile_pool(name="sqpool2", bufs=1) as sq_pool2,
        tc.tile_pool(name="lnps2", bufs=1, space="PSUM") as ln_ps2,
    ):
        xsq2 = []
        for m in range(NJ):
            pt = mm_ps2.tile([P, TO], F32, tag="mm", name=f"mmproj{m}")
            for j in range(NJ):
                nc.tensor.matmul(pt[:], lhsT=wp[j][:, m * P:(m + 1) * P],
                                 rhs=attn_fm[j],
                                 start=(j == 0), stop=(j == NJ - 1))
            nc.vector.scalar_tensor_tensor(
                out=R(x2_sb[m]), in0=pt, scalar=bproj_sb[:, m:m + 1],
                in1=xt0[m], op0=ALU.add, op1=ALU.add)
            t = sq_pool2.tile([P, TO], F32, tag=f"x2sq{m}", name="x2sqt")
            nc.scalar.activation(out=R(t), in_=x2_sb[m], func=AF.Square)
            xsq2.append(t)
        ps_sum = ln_ps2.tile([2, TO], F32, tag="lnsum2", name="ps_sum2")
        ps_sq = ln_ps2.tile([2, TO], F32, tag="lnsq2", name="ps_sq2")
        for j in range(NJ):
            nc.tensor.matmul(ps_sum[:], lhsT=R(ones2), rhs=R(x2_sb[j]),
                             start=(j == 0), stop=(j == NJ - 1))
        for j in range(NJ):
            nc.tensor.matmul(ps_sq[:], lhsT=R(ones2), rhs=R(xsq2[j]),
                             start=(j == 0), stop=(j == NJ - 1))
        sum2 = cons_pool.tile([2, TO], F32, tag="sum2", name="sum2")
        sq2 = cons_pool.tile([2, TO], F32, tag="sq2", name="sq2")
        rs2 = cons_pool.tile([2, TO], F32, tag="rs2", name="rs2")
        nc.vector.tensor_copy(out=R(sum2), in_=ps_sum)
        nc.vector.tensor_copy(out=sq2, in_=ps_sq)
        nc.vector.scalar_tensor_tensor(out=R(rs2), in0=sum2, scalar=-1.0,
                                       in1=sum2, op0=ALU.mult, op1=ALU.mult)
        nc.vector.scalar_tensor_tensor(out=sq2, in0=sq2, scalar=float(D),
                                       in1=rs2, op0=ALU.mult, op1=ALU.add)
        nc.scalar.activation(out=sq2, in_=sq2, func=AF.Ln, bias=eps2,
                             scale=1.0 / (D * D))
        nc.scalar.activation(out=R(rs2), in_=sq2, func=AF.Exp, scale=-0.5)
        nc.vector.scalar_tensor_tensor(out=R(sum2), in0=sum2,
                                       scalar=-1.0 / D,
                                       in1=rs2, op0=ALU.mult, op1=ALU.mult)
        ps_a = ln_ps2.tile([P, TO], F32, tag="bca2", name="ps_a2")
        nc.tensor.matmul(ps_a[:], lhsT=R(half2), rhs=R(rs2),
                         start=True, stop=True)
        ps_c = ln_ps2.tile([P, TO], F32, tag="bcc2", name="ps_c2")
        nc.tensor.matmul(ps_c[:], lhsT=R(half2), rhs=R(sum2),
                         start=True, stop=True)
        for j in range(NJ):
            tmp = sq_pool2.tile([P, TO], F32, tag=f"tmp2{j}", name="h2t")
            nc.vector.tensor_mul(out=tmp, in0=x2_sb[j], in1=ps_a)
            nc.vector.tensor_add(out=h_sb[j], in0=tmp, in1=ps_c)
    wp_pool.release()
    attn_pool.release()
    persist.release()
    x0_pool.release()

    # ---------------- fc1 + gelu -> h1 (bf16) ----------------
    h1_pool = tc.alloc_tile_pool(name="h1", bufs=1, side="right")
    ctx_pools.append(h1_pool)
    h1_sb = [h1_pool.tile([P, TO], BF16, tag=f"h1{m}", name=f"h1{m}")
             for m in range(NMLP)]
    with (
        tc.tile_pool(name="mmps3", bufs=4, space="PSUM") as mm_ps3,
    ):
        for m in range(NMLP):
            pt = mm_ps3.tile([P, TO], F32, tag="mm", name="mmfc1")
            for j in range(NJ):
                nc.tensor.matmul(pt[:], lhsT=wf1[j][:, m * P:(m + 1) * P],
                                 rhs=h_sb[j],
                                 start=(j == 0), stop=(j == NJ - 1))
            nc.scalar.activation(out=h1_sb[m], in_=pt, func=AF.Gelu,
                                 bias=bfc1_sb[:, m:m + 1])

    # ---------------- fc2 + residual + store ----------------
    with (
        tc.tile_pool(name="wfc2", bufs=6) as wfc2_pool,
        tc.tile_pool(name="fc2ps", bufs=1, space="PSUM") as fc2_ps,
        tc.tile_pool(name="outsb", bufs=2) as out_pool,
    ):
        pts = [fc2_ps.tile([P, TO], F32, tag=f"fc2_{m}", name=f"fc2_{m}")
               for m in range(NJ)]
        for j in range(NMLP):
            wt = wfc2_pool.tile([P, D], BF16, tag="wf2", name="wf2")
            nc.sync.dma_start(out=wt, in_=wfc2_d[j * P:(j + 1) * P, :])
            for m in range(NJ):
                nc.tensor.matmul(pts[m][:], lhsT=wt[:, m * P:(m + 1) * P],
                                 rhs=h1_sb[j],
                                 start=(j == 0), stop=(j == NMLP - 1))
        for m in range(NJ):
            ot = out_pool.tile([P, TO], F32, tag="out", name="ot")
            nc.vector.scalar_tensor_tensor(
                out=ot, in0=pts[m], scalar=bfc2_sb[:, m:m + 1],
                in1=x2_sb[m], op0=ALU.add, op1=ALU.add)
            nc.sync.dma_start(out=out_fm[m * P:(m + 1) * P, :], in_=ot)

    for pool in reversed(ctx_pools):
        pool.release()


_NC_CACHE = {}


def _get_nc():
    if "nc" not in _NC_CACHE:
        _NC_CACHE["nc"] = _build()
    return _NC_CACHE["nc"]


def _host_prep(inputs):
    f32 = lambda a: np.ascontiguousarray(np.asarray(a, dtype=np.float32))
    x = f32(inputs["x"])            # [2, 2048, 768]
    W_qkv = f32(inputs["W_qkv"])    # [768, 2304]
    b_qkv = f32(inputs["b_qkv"])
    W_proj = f32(inputs["W_proj"])
    b_proj = f32(inputs["b_proj"])
    W_fc1 = f32(inputs["W_fc1"])
    b_fc1 = f32(inputs["b_fc1"])
    W_fc2 = f32(inputs["W_fc2"])
    b_fc2 = f32(inputs["b_fc2"])
    ln1_g = f32(inputs["ln1_g"])
    ln1_b = f32(inputs["ln1_b"])
    ln2_g = f32(inputs["ln2_g"])
    ln2_b = f32(inputs["ln2_b"])

    scale = DH ** -0.5
    wqkv_eff = W_qkv * ln1_g[:, None]
    bqkv_eff = ln1_b @ W_qkv + b_qkv
    wqkv_eff[:, :D] *= scale
    bqkv_eff_q = bqkv_eff[:D] * scale
    bqk = np.concatenate([bqkv_eff_q, bqkv_eff[D:2 * D]]).astype(np.float32)
    bv = bqkv_eff[2 * D:]
    bproj_eff = (b_proj + bv @ W_proj).astype(np.float32)
    wfc1_eff = (W_fc1 * ln2_g[:, None]).astype(np.float32)
    bfc1_eff = (ln2_b @ W_fc1 + b_fc1).astype(np.float32)

    # Selection constants for softmax-denominator gather / broadcast.
    selg = np.zeros((VW, 24 * NH), dtype=np.float32)
    selbc = np.zeros((2 * NH, 64 * NH), dtype=np.float32)
    for h in range(NH):
        selg[64:66, 24 * h + 2 * h:24 * h + 2 * h + 2] = np.eye(2)
        selbc[2 * h:2 * h + 2, h * 64:(h + 1) * 64] = 0.5
    selg = selg.astype(ml_dtypes.bfloat16)

    bf = lambda a: np.ascontiguousarray(a.astype(ml_dtypes.bfloat16))
    pack = lambda b: np.ascontiguousarray(
        b.reshape(-1, P).T.astype(np.float32))
    shared = {
        "selg": selg,
        "selbc": selbc,
        "wqkv": bf(wqkv_eff),
        "bqk": pack(bqk),
        "wproj": bf(W_proj),
        "bproj": pack(bproj_eff),
        "wfc1": bf(wfc1_eff),
        "bfc1": pack(bfc1_eff),
        "wfc2": bf(W_fc2),
        "bfc2": pack(b_fc2),
    }
    in_maps = []
    for c in range(N_CORES):
        b, q = divmod(c, 4)
        xb = np.roll(x[b], -TO * q, axis=0)  # own tokens at rows 0:TO
        m = dict(shared)
        m["x_fm"] = bf(xb.T)
        in_maps.append(m)
    return in_maps


def _run(inputs, trace=False):
    nc = _get_nc()
    in_maps = _host_prep(inputs)
    res = bass_utils.run_bass_kernel_spmd(nc, in_maps, list(range(N_CORES)),
                                          trace=trace)
    B = 2
    out = np.empty((B, TB, D), dtype=np.float32)
    for c in range(N_CORES):
        b, q = divmod(c, 4)
        out[b, TO * q:TO * (q + 1), :] = res.results[c]["out_fm"].T
    return out, res


def kernel(**inputs):
    out, _ = _run(inputs, trace=False)
    return out


if __name__ == "__main__":
    print("building...")
    _get_nc()
    print("built ok")
